# revision 9
# baseline (speedup 1.0000x reference)
"""Trainium2 Bass kernel for AdditiveAttention (per-batch bmm attention).

Full computation per batch element b (x: (C, N) with C=256, N=48*48=2304):
    q = Wq @ x + bq            (KC=32, N)
    k = Wk @ x + bk            (KC, N)
    v = Wv @ x + bv            (C, N)
    s = (q^T k) / sqrt(KC)     (N, N)
    a = softmax(s, axis=-1)
    out = v @ a^T              (C, N)
    y = gamma * out + x

Distribution: data-parallel over batch B=16 across 8 cores (2 per core);
the small channel-mixing weights are replicated.

Device-side layout strategy (per batch element):
  - Everything is computed with the attention "scores" TRANSPOSED
    (j on partitions, i on free dim), which makes every matmul feed the
    next one without explicit transposes:
      scoresT[j, i] = sum_kc k[kc, j] q[kc, i]   (lhsT = k slice, rhs = q)
      e = exp(scoresT / sqrt(KC))                (elementwise, layout-free)
      outT[i, c]   = sum_j e[j, i] vT[j, c]      (lhsT = e chunk, rhs = vT)
  - The softmax denominator comes for free from a ones-column appended to
    vT: outT[:, C] = sum_j e[j, i] = rowsum[i], a per-partition scalar in
    the outT layout, so normalization is a native per-partition multiply.
  - gamma and bv are folded into the V weights on the host (y = gamma*out + x
    with out = (gamma*Wv) x + gamma*bv, normalized by the unscaled rowsum).
  - The final (i, c) -> (c, i) layout flip uses PE transposes, then the
    residual x is added in fp32 and stored.

Pipelining: the exp stage is ScalarE-bound while the outT stage is
TensorE-bound, so the i-axis is split into 3 chunks of 768 and the emission
is software-pipelined at j/group granularity: outT groups for chunk t are
interleaved with scores+exp for chunk t+1, and each group's normalize/
transpose/residual epilogue is deferred by one group so TensorE never waits
on the VectorE round-trip. This keeps TensorE dense (no HAM re-throttle).
"""

import math
import time
from contextlib import ExitStack

import numpy as np
import ml_dtypes

import concourse.bass as bass
import concourse.bacc as bacc
import concourse.mybir as mybir
import concourse.tile as tile
from concourse.bass_utils import run_bass_kernel_spmd
from concourse.masks import make_identity

B, C, KC, H, W = 16, 256, 32, 48, 48
N = H * W            # 2304
NCORES = 8
BPC = B // NCORES    # batch elements per core = 2
P = 128
NB = N // P          # 18 n-blocks
CB = C // P          # 2 channel chunks
NT = 3               # i-chunks per batch element (pipeline stages)
TW = N // NT         # 768 chunk width
IBT = NB // NT       # 6 i-blocks per chunk

F32 = mybir.dt.float32
BF16 = mybir.dt.bfloat16
EXP = mybir.ActivationFunctionType.Exp


class _Builder:
    def __init__(self):
        nc = bacc.Bacc()
        self.nc = nc
        self.xb = nc.dram_tensor("xb", [BPC, CB, P, N], BF16, kind="ExternalInput")
        self.xf = nc.dram_tensor("xf", [BPC, CB, P, N], F32, kind="ExternalInput")
        self.wq = nc.dram_tensor("wq", [P, CB, KC], BF16, kind="ExternalInput")
        self.wk = nc.dram_tensor("wk", [P, CB, KC], BF16, kind="ExternalInput")
        self.wv = nc.dram_tensor("wv", [P, CB, C], BF16, kind="ExternalInput")
        self.bq = nc.dram_tensor("bq", [KC, 1], F32, kind="ExternalInput")
        self.bk = nc.dram_tensor("bk", [KC, 1], F32, kind="ExternalInput")
        self.bv = nc.dram_tensor("bv", [1, C], BF16, kind="ExternalInput")
        self.y = nc.dram_tensor("y", [BPC, CB, P, N], F32, kind="ExternalOutput")
        self.scale = 1.0 / math.sqrt(KC)
        self.pending = []  # deferred outT epilogues

    def build(self):
        nc = self.nc
        with tile.TileContext(nc) as tc, ExitStack() as ctx:
            self.tc = tc
            const = ctx.enter_context(tc.tile_pool(name="const", bufs=1))
            self.xpool = ctx.enter_context(tc.tile_pool(name="xpool", bufs=2 * CB))
            self.qkpool = ctx.enter_context(tc.tile_pool(name="qkpool", bufs=2))
            self.epool = ctx.enter_context(tc.tile_pool(name="epool", bufs=NT * NB))
            self.vpool = ctx.enter_context(tc.tile_pool(name="vpool", bufs=2 * NB))
            self.ntpool = ctx.enter_context(tc.tile_pool(name="ntpool", bufs=3))
            self.xrpool = ctx.enter_context(tc.tile_pool(name="xrpool", bufs=6))
            self.ypool = ctx.enter_context(tc.tile_pool(name="ypool", bufs=3))
            self.rpool = ctx.enter_context(tc.tile_pool(name="rpool", bufs=3))
            self.ps_s = ctx.enter_context(tc.tile_pool(name="ps_s", bufs=2, space="PSUM"))
            self.ps_o = ctx.enter_context(tc.tile_pool(name="ps_o", bufs=2, space="PSUM"))
            self.ps_t = ctx.enter_context(tc.tile_pool(name="ps_t", bufs=2, space="PSUM"))

            warm = const.tile([1, 2], F32)
            nc.vector.memset(warm, 0.0)
            nc.scalar.activation(out=warm, in_=warm, func=EXP)
            self.ident = const.tile([P, P], BF16)
            make_identity(nc, self.ident)

            self.wq_sb = const.tile([P, CB, KC], BF16)
            nc.sync.dma_start(out=self.wq_sb, in_=self.wq[:])
            self.wk_sb = const.tile([P, CB, KC], BF16)
            nc.sync.dma_start(out=self.wk_sb, in_=self.wk[:])
            self.wv_sb = const.tile([P, CB, C], BF16)
            nc.sync.dma_start(out=self.wv_sb, in_=self.wv[:])
            self.bq_sb = const.tile([KC, 1], F32)
            nc.sync.dma_start(out=self.bq_sb, in_=self.bq[:])
            self.bk_sb = const.tile([KC, 1], F32)
            nc.sync.dma_start(out=self.bk_sb, in_=self.bk[:])
            # bv broadcast to all partitions so it can be added on the
            # psum->sbuf copy of vT (per-free-element bias)
            self.bvb_sb = const.tile([P, C], BF16)
            nc.sync.dma_start(out=self.bvb_sb, in_=self.bv[:].to_broadcast([P, C]))

            # per-b state
            self.xs = {}      # b -> [x tile per cc]
            self.q = {}       # b -> q tile (KC, N)
            self.k = {}
            self.vts = {}     # b -> [vt tile per nb]
            self.es = {}      # (b, t) -> [e tile per j], each (P, TW)

            self.emit_x_alloc(0)
            self.emit_x_alloc(1)
            for ic in range(3):
                i0 = ic * 1024
                sz = min(1024, N - i0)
                self.emit_x_load_chunk(0, i0, sz)
                self.emit_qk_chunk(0, 2 * ic)
                self.emit_qk_chunk(0, 2 * ic + 1)
            self.emit_x_load_chunk(1, 0, 1024)
            self.emit_x_load_chunk(1, 1024, 1024)
            self.emit_x_load_chunk(1, 2048, 256)
            # S1 (prologue): exp chunk (0,0) interleaved with vT(0) and vT(1)
            for j in range(NB):
                self.emit_scores_exp_j(0, 0, j)
                self.emit_vt_one(0, j)
                self.emit_vt_one(1, j)
            # steady-state pipeline
            self.emit_stage((0, 1), (0, 0), extra="qk1")
            self.emit_stage((0, 2), (0, 1))
            self.emit_stage((1, 0), (0, 2))
            self.emit_stage((1, 1), (1, 0))
            self.emit_stage((1, 2), (1, 1))
            self.emit_stage(None, (1, 2))
            while self.pending:
                self.emit_epi()

        nc.finalize()
        return nc

    def emit_x_alloc(self, b):
        self.xs[b] = [self.xpool.tile([P, N], BF16, tag="xsb", name="x_sb") for _ in range(CB)]

    def emit_x_load_chunk(self, b, i0, sz):
        for cc in range(CB):
            self.nc.sync.dma_start(
                out=self.xs[b][cc][:, i0 : i0 + sz], in_=self.xb[b, cc, :, i0 : i0 + sz]
            )

    def emit_qk_chunk(self, b, g):
        """One of 6 q/k psum chunks: g even -> q, g odd -> k; i0 = (g//2)*1024."""
        nc = self.nc
        if b not in self.q:
            self.q[b] = self.qkpool.tile([KC, N], BF16, tag="q", name="q_sb")
            self.k[b] = self.qkpool.tile([KC, N], BF16, tag="k", name="k_sb")
        dst, w_sb, b_sb = (
            (self.q[b], self.wq_sb, self.bq_sb)
            if g % 2 == 0
            else (self.k[b], self.wk_sb, self.bk_sb)
        )
        i0 = (g // 2) * 1024
        sz = min(1024, N - i0)
        ps = self.ps_s.tile([P, 1024], F32, tag="ps_s")
        for s0 in range(0, sz, 512):
            s1 = min(512, sz - s0)
            for cc in range(CB):
                nc.tensor.matmul(
                    ps[0:KC, s0 : s0 + s1],
                    lhsT=w_sb[:, cc, :],
                    rhs=self.xs[b][cc][:, i0 + s0 : i0 + s0 + s1],
                    start=(cc == 0),
                    stop=(cc == CB - 1),
                )
        nc.vector.tensor_scalar_add(dst[:, i0 : i0 + sz], ps[0:KC, 0:sz], b_sb)

    def emit_vt_one(self, b, nb):
        nc = self.nc
        if b not in self.vts:
            self.vts[b] = [None] * NB
        ps = self.ps_t.tile([P, 2 * P], F32, tag="ps_t")
        for cc in range(CB):
            nc.tensor.matmul(
                ps[:, 0:C],
                lhsT=self.xs[b][cc][:, nb * P : (nb + 1) * P],
                rhs=self.wv_sb[:, cc, :],
                start=(cc == 0),
                stop=(cc == CB - 1),
            )
        vt = self.vpool.tile([P, C + 1], BF16, tag="vt")
        nc.vector.tensor_add(vt[:, 0:C], ps[:, 0:C], self.bvb_sb)
        nc.gpsimd.memset(vt[:, C : C + 1], 1.0)
        self.vts[b][nb] = vt

    def emit_scores_exp_j(self, b, t, j):
        """scoresT chunk (P j-rows, TW i-cols) + exp -> e tile."""
        nc = self.nc
        i0 = t * TW
        ps = self.ps_s.tile([P, 1024], F32, tag="ps_s")
        for s0 in range(0, TW, 512):
            s1 = min(512, TW - s0)
            nc.tensor.matmul(
                ps[:, s0 : s0 + s1],
                lhsT=self.k[b][:, j * P : (j + 1) * P],
                rhs=self.q[b][:, i0 + s0 : i0 + s0 + s1],
                start=True,
                stop=True,
            )
        e = self.epool.tile([P, TW], BF16, tag="e")
        nc.scalar.activation(out=e, in_=ps[:, 0:TW], func=EXP, scale=self.scale)
        self.es.setdefault((b, t), []).append(e)

    def emit_outT_accum(self, b, t, g):
        """outT accumulation for i-block ib = t*IBT + g; epilogue deferred."""
        nc = self.nc
        ib = t * IBT + g
        es = self.es[(b, t)]
        po = self.ps_o.tile([P, C + 1], F32, tag="ps_o")
        for j in range(NB):
            nc.tensor.matmul(
                po,
                lhsT=es[j][:, g * P : (g + 1) * P],
                rhs=self.vts[b][j],
                start=(j == 0),
                stop=(j == NB - 1),
            )
        # prefetch the residual x chunks for this i-block
        xrts = []
        for ch in range(CB):
            xrt = self.xrpool.tile([P, P], F32, tag="xr")
            nc.sync.dma_start(out=xrt, in_=self.xf[b, ch, :, ib * P : (ib + 1) * P])
            xrts.append(xrt)
        self.pending.append((b, ib, po, xrts))

    def emit_epi(self):
        """Normalize + transpose + residual + store for the oldest group."""
        nc = self.nc
        b, ib, po, xrts = self.pending.pop(0)
        rec = self.rpool.tile([P, 1], F32, tag="rec")
        nc.vector.reciprocal(rec, po[:, C : C + 1])
        nt = self.ntpool.tile([P, C], BF16, tag="nt")
        nc.vector.tensor_scalar_mul(nt, po[:, 0:C], rec)
        tp = self.ps_t.tile([P, 2 * P], BF16, tag="ps_t", name="tp")
        for ch in range(CB):
            nc.tensor.transpose(tp[:, ch * P : (ch + 1) * P], nt[:, ch * P : (ch + 1) * P], self.ident)
        ys = self.ypool.tile([P, CB, P], F32, tag="ys")
        for ch in range(CB):
            nc.vector.tensor_add(ys[:, ch, :], tp[:, ch * P : (ch + 1) * P], xrts[ch])
        nc.sync.dma_start(
            out=self.y[b].rearrange("t p n -> p t n")[:, :, ib * P : (ib + 1) * P],
            in_=ys,
        )

    def emit_stage(self, a, b_, extra=None):
        """One pipeline stage: exp-chunk `a` interleaved with outT-chunk `b_`."""
        for g in range(IBT):
            if b_ is not None:
                self.emit_outT_accum(b_[0], b_[1], g)
            if a is not None:
                for jj in range(3 * g, 3 * g + 3):
                    self.emit_scores_exp_j(a[0], a[1], jj)
            if extra == "qk1":
                self.emit_qk_chunk(1, g)
            elif extra == "vt1":
                for nb in range(3 * g, 3 * g + 3):
                    self.emit_vt_one(1, nb)
            if b_ is not None and len(self.pending) >= 2:
                self.emit_epi()


def _build_nc():
    return _Builder().build()


_CACHE = {}


def kernel(x, Wq, bq, Wk, bk, Wv, bv, gamma):
    x = np.asarray(x, dtype=np.float32)
    Wq = np.asarray(Wq, dtype=np.float32)
    bq = np.asarray(bq, dtype=np.float32)
    Wk = np.asarray(Wk, dtype=np.float32)
    bk = np.asarray(bk, dtype=np.float32)
    Wv = np.asarray(Wv, dtype=np.float32)
    bv = np.asarray(bv, dtype=np.float32)
    gamma = np.asarray(gamma, dtype=np.float32)
    g = float(gamma[0])

    xfull = x.reshape(B, C, N)
    # (B, C, N) -> (NCORES, BPC, CB, P, N)
    xblk = xfull.reshape(NCORES, BPC, CB, P, N)

    def chan_block(w):  # (C, K) -> (P, CB, K), partition-major channel blocking
        ck = w.shape[1]
        return np.ascontiguousarray(w.reshape(CB, P, ck).transpose(1, 0, 2))

    wq_h = chan_block(Wq.T).astype(ml_dtypes.bfloat16)          # (P, CB, KC)
    wk_h = chan_block(Wk.T).astype(ml_dtypes.bfloat16)          # (P, CB, KC)
    wv_h = chan_block((Wv * g).T).astype(ml_dtypes.bfloat16)    # (P, CB, C): (g*Wv)^T
    bq_h = np.ascontiguousarray(bq.reshape(KC, 1))
    bk_h = np.ascontiguousarray(bk.reshape(KC, 1))
    bv_h = (bv * g).reshape(1, C).astype(ml_dtypes.bfloat16)

    if "nc" not in _CACHE:
        _CACHE["nc"] = _build_nc()
    nc = _CACHE["nc"]

    in_maps = []
    for core in range(NCORES):
        xc = np.ascontiguousarray(xblk[core])
        in_maps.append(
            {
                "xb": xc.astype(ml_dtypes.bfloat16),
                "xf": xc,
                "wq": wq_h,
                "wk": wk_h,
                "wv": wv_h,
                "bq": bq_h,
                "bk": bk_h,
                "bv": bv_h,
            }
        )

    res = run_bass_kernel_spmd(nc, in_maps, core_ids=list(range(NCORES)))
    out = np.stack([res.results[i]["y"] for i in range(NCORES)])
    # (NCORES, BPC, CB, P, N) -> (B, C, H, W)
    return np.ascontiguousarray(out.reshape(B, C, H, W))


if __name__ == "__main__":
    t0 = time.time()
    nc = _build_nc()
    print(f"build ok: {time.time() - t0:.1f}s")


# revision 13
# speedup vs baseline: 1.1730x; 1.1730x over previous
"""Trainium2 Bass kernel for AdditiveAttention (per-batch bmm attention).

Full computation per batch element b (x: (C, N) with C=256, N=48*48=2304):
    q = Wq @ x + bq            (KC=32, N)
    k = Wk @ x + bk            (KC, N)
    v = Wv @ x + bv            (C, N)
    s = (q^T k) / sqrt(KC)     (N, N)
    a = softmax(s, axis=-1)
    out = v @ a^T              (C, N)
    y = gamma * out + x

Distribution: data-parallel over batch B=16 across 8 cores (2 per core);
the small channel-mixing weights are replicated.

Device-side layout strategy (per batch element):
  - Everything is computed with the attention "scores" TRANSPOSED
    (j on partitions, i on free dim), which makes every matmul feed the
    next one without explicit transposes:
      scoresT[j, i] = sum_kc k[kc, j] q[kc, i]   (lhsT = k slice, rhs = q)
      e = exp(scoresT / sqrt(KC))                (elementwise, layout-free)
      outT[i, c]   = sum_j e[j, i] vT[j, c]      (lhsT = e chunk, rhs = vT)
  - The softmax denominator comes for free from a ones-column appended to
    vT: outT[:, C] = sum_j e[j, i] = rowsum[i], a per-partition scalar in
    the outT layout, so normalization is a native per-partition multiply.
  - gamma and bv are folded into the V weights on the host (y = gamma*out + x
    with out = (gamma*Wv) x + gamma*bv, normalized by the unscaled rowsum).
  - The final (i, c) -> (c, i) layout flip uses PE transposes, then the
    residual x is added in fp32 and stored.

Pipelining: the exp stage is ScalarE-bound while the outT stage is
TensorE-bound, so the i-axis is split into 3 chunks of 768 and the emission
is software-pipelined at j/group granularity: outT groups for chunk t are
interleaved with scores+exp for chunk t+1, and each group's normalize/
transpose/residual epilogue is deferred by one group so TensorE never waits
on the VectorE round-trip. This keeps TensorE dense (no HAM re-throttle).
"""

import math
import time
from contextlib import ExitStack

import numpy as np
import ml_dtypes

import concourse.bass as bass
import concourse.bacc as bacc
import concourse.mybir as mybir
import concourse.tile as tile
from concourse.bass_utils import run_bass_kernel_spmd
from concourse.masks import make_identity

B, C, KC, H, W = 16, 256, 32, 48, 48
N = H * W            # 2304
NCORES = 8
BPC = B // NCORES    # batch elements per core = 2
P = 128
NB = N // P          # 18 n-blocks
CB = C // P          # 2 channel chunks
NT = 3               # i-chunks per batch element (pipeline stages)
TW = N // NT         # 768 chunk width
IBT = NB // NT       # 6 i-blocks per chunk

F32 = mybir.dt.float32
BF16 = mybir.dt.bfloat16
F8 = mybir.dt.float8e4
DR = mybir.MatmulPerfMode.DoubleRow
EXP = mybir.ActivationFunctionType.Exp
EXP_SHIFT = -2.5  # exp(s/sqrt(KC) - 2.5): keeps e in fp8e4m3 range; cancels in softmax


class _Builder:
    def __init__(self):
        nc = bacc.Bacc()
        self.nc = nc
        self.xb = nc.dram_tensor("xb", [BPC, CB, P, N], BF16, kind="ExternalInput")
        self.xf = nc.dram_tensor("xf", [BPC, CB, P, N], F32, kind="ExternalInput")
        self.wq = nc.dram_tensor("wq", [P, CB, KC], BF16, kind="ExternalInput")
        self.wk = nc.dram_tensor("wk", [P, CB, KC], BF16, kind="ExternalInput")
        self.wv = nc.dram_tensor("wv", [P, CB, C], BF16, kind="ExternalInput")
        self.bq = nc.dram_tensor("bq", [KC, 1], F32, kind="ExternalInput")
        self.bk = nc.dram_tensor("bk", [KC, 1], F32, kind="ExternalInput")
        self.bv = nc.dram_tensor("bv", [1, C], BF16, kind="ExternalInput")
        self.y = nc.dram_tensor("y", [BPC, CB, P, N], F32, kind="ExternalOutput")
        self.scale = 1.0 / math.sqrt(KC)
        self.pending = []  # deferred outT epilogues

    def build(self):
        nc = self.nc
        with tile.TileContext(nc) as tc, ExitStack() as ctx:
            self.tc = tc
            const = ctx.enter_context(tc.tile_pool(name="const", bufs=1))
            self.xpool = ctx.enter_context(tc.tile_pool(name="xpool", bufs=2 * CB))
            self.qkpool = ctx.enter_context(tc.tile_pool(name="qkpool", bufs=2))
            self.epool = ctx.enter_context(tc.tile_pool(name="epool", bufs=NT * NB))
            self.vpool = ctx.enter_context(tc.tile_pool(name="vpool", bufs=2 * NB))
            self.ntpool = ctx.enter_context(tc.tile_pool(name="ntpool", bufs=3))
            self.xrpool = ctx.enter_context(tc.tile_pool(name="xrpool", bufs=6))
            self.ypool = ctx.enter_context(tc.tile_pool(name="ypool", bufs=3))
            self.rpool = ctx.enter_context(tc.tile_pool(name="rpool", bufs=3))
            self.ps_s = ctx.enter_context(tc.tile_pool(name="ps_s", bufs=2, space="PSUM"))
            self.ps_o = ctx.enter_context(tc.tile_pool(name="ps_o", bufs=2, space="PSUM"))
            self.ps_t = ctx.enter_context(tc.tile_pool(name="ps_t", bufs=2, space="PSUM"))

            warm = const.tile([1, 2], F32)
            nc.vector.memset(warm, 0.0)
            self.eshift = const.tile([P, 1], F32)
            nc.vector.memset(self.eshift, EXP_SHIFT)
            nc.scalar.activation(out=warm, in_=warm, func=EXP)
            self.ident = const.tile([P, P], BF16)
            make_identity(nc, self.ident)

            self.wq_sb = const.tile([P, CB, KC], BF16)
            nc.sync.dma_start(out=self.wq_sb, in_=self.wq[:])
            self.wk_sb = const.tile([P, CB, KC], BF16)
            nc.sync.dma_start(out=self.wk_sb, in_=self.wk[:])
            self.wv_sb = const.tile([P, CB, C], BF16)
            nc.sync.dma_start(out=self.wv_sb, in_=self.wv[:])
            self.bq_sb = const.tile([KC, 1], F32)
            nc.sync.dma_start(out=self.bq_sb, in_=self.bq[:])
            self.bk_sb = const.tile([KC, 1], F32)
            nc.sync.dma_start(out=self.bk_sb, in_=self.bk[:])
            # bv broadcast to all partitions so it can be added on the
            # psum->sbuf copy of vT (per-free-element bias)
            self.bvb_sb = const.tile([P, C], BF16)
            nc.sync.dma_start(out=self.bvb_sb, in_=self.bv[:].to_broadcast([P, C]))

            # per-b state
            self.xs = {}      # b -> [x tile per cc]
            self.q = {}       # b -> q tile (KC, N)
            self.k = {}
            self.vts = {}     # b -> [vt tile per nb]
            self.es = {}      # (b, t) -> [e tile per j], each (P, TW)

            self.emit_x_alloc(0)
            self.emit_x_alloc(1)
            for ic in range(3):
                i0 = ic * 1024
                sz = min(1024, N - i0)
                self.emit_x_load_chunk(0, i0, sz)
                self.emit_qk_chunk(0, 2 * ic)
                self.emit_qk_chunk(0, 2 * ic + 1)
            self.emit_x_load_chunk(1, 0, 1024)
            self.emit_x_load_chunk(1, 1024, 1024)
            self.emit_x_load_chunk(1, 2048, 256)
            # S1 (prologue): exp chunk (0,0) interleaved with vT(0) and vT(1)
            for j in range(NB):
                self.emit_scores_exp_j(0, 0, j)
                self.emit_vt_one(0, j)
                self.emit_vt_one(1, j)
            # steady-state pipeline
            self.emit_stage((0, 1), (0, 0), extra="qk1")
            self.emit_stage((0, 2), (0, 1))
            self.emit_stage((1, 0), (0, 2))
            self.emit_stage((1, 1), (1, 0))
            self.emit_stage((1, 2), (1, 1))
            self.emit_stage(None, (1, 2))
            while self.pending:
                self.emit_epi()

        nc.finalize()
        return nc

    def emit_x_alloc(self, b):
        self.xs[b] = [self.xpool.tile([P, N], BF16, tag="xsb", name="x_sb") for _ in range(CB)]

    def emit_x_load_chunk(self, b, i0, sz):
        for cc in range(CB):
            self.nc.sync.dma_start(
                out=self.xs[b][cc][:, i0 : i0 + sz], in_=self.xb[b, cc, :, i0 : i0 + sz]
            )

    def emit_qk_chunk(self, b, g):
        """One of 6 q/k psum chunks: g even -> q, g odd -> k; i0 = (g//2)*1024."""
        nc = self.nc
        if b not in self.q:
            self.q[b] = self.qkpool.tile([KC, N], BF16, tag="q", name="q_sb")
            self.k[b] = self.qkpool.tile([KC, N], BF16, tag="k", name="k_sb")
        dst, w_sb, b_sb = (
            (self.q[b], self.wq_sb, self.bq_sb)
            if g % 2 == 0
            else (self.k[b], self.wk_sb, self.bk_sb)
        )
        i0 = (g // 2) * 1024
        sz = min(1024, N - i0)
        ps = self.ps_s.tile([P, 1024], F32, tag="ps_s")
        for s0 in range(0, sz, 512):
            s1 = min(512, sz - s0)
            for cc in range(CB):
                nc.tensor.matmul(
                    ps[0:KC, s0 : s0 + s1],
                    lhsT=w_sb[:, cc, :],
                    rhs=self.xs[b][cc][:, i0 + s0 : i0 + s0 + s1],
                    start=(cc == 0),
                    stop=(cc == CB - 1),
                )
        nc.vector.tensor_scalar_add(dst[:, i0 : i0 + sz], ps[0:KC, 0:sz], b_sb)

    def emit_vt_one(self, b, nb):
        nc = self.nc
        if b not in self.vts:
            self.vts[b] = [None] * NB
        ps = self.ps_t.tile([P, 2 * P], F32, tag="ps_t")
        for cc in range(CB):
            nc.tensor.matmul(
                ps[:, 0:C],
                lhsT=self.xs[b][cc][:, nb * P : (nb + 1) * P],
                rhs=self.wv_sb[:, cc, :],
                start=(cc == 0),
                stop=(cc == CB - 1),
            )
        vt = self.vpool.tile([P, C + 1], BF16, tag="vt")
        nc.vector.tensor_add(vt[:, 0:C], ps[:, 0:C], self.bvb_sb)
        nc.gpsimd.memset(vt[:, C : C + 1], 1.0)
        self.vts[b][nb] = vt

    def emit_scores_exp_j(self, b, t, j):
        """scoresT chunk (P j-rows, TW i-cols) + exp -> e tile."""
        nc = self.nc
        i0 = t * TW
        ps = self.ps_s.tile([P, 1024], F32, tag="ps_s")
        for s0 in range(0, TW, 512):
            s1 = min(512, TW - s0)
            nc.tensor.matmul(
                ps[:, s0 : s0 + s1],
                lhsT=self.k[b][:, j * P : (j + 1) * P],
                rhs=self.q[b][:, i0 + s0 : i0 + s0 + s1],
                start=True,
                stop=True,
            )
        e = self.epool.tile([P, TW], BF16, tag="e")
        nc.scalar.activation(out=e, in_=ps[:, 0:TW], func=EXP, scale=self.scale)
        self.es.setdefault((b, t), []).append(e)

    def emit_outT_accum(self, b, t, g):
        """outT accumulation for i-block ib = t*IBT + g; epilogue deferred."""
        nc = self.nc
        ib = t * IBT + g
        es = self.es[(b, t)]
        po = self.ps_o.tile([P, C + 1], F32, tag="ps_o")
        for j in range(NB):
            nc.tensor.matmul(
                po,
                lhsT=es[j][:, g * P : (g + 1) * P],
                rhs=self.vts[b][j],
                start=(j == 0),
                stop=(j == NB - 1),
            )
        # prefetch the residual x chunks for this i-block
        xrts = []
        for ch in range(CB):
            xrt = self.xrpool.tile([P, P], F32, tag="xr")
            nc.sync.dma_start(out=xrt, in_=self.xf[b, ch, :, ib * P : (ib + 1) * P])
            xrts.append(xrt)
        self.pending.append((b, ib, po, xrts))

    def emit_epi(self):
        """Normalize + transpose + residual + store for the oldest group."""
        nc = self.nc
        b, ib, po, xrts = self.pending.pop(0)
        rec = self.rpool.tile([P, 1], F32, tag="rec")
        nc.vector.reciprocal(rec, po[:, C : C + 1])
        nt = self.ntpool.tile([P, C], BF16, tag="nt")
        nc.vector.tensor_scalar_mul(nt, po[:, 0:C], rec)
        tp = self.ps_t.tile([P, 2 * P], BF16, tag="ps_t", name="tp")
        for ch in range(CB):
            nc.tensor.transpose(tp[:, ch * P : (ch + 1) * P], nt[:, ch * P : (ch + 1) * P], self.ident)
        ys = self.ypool.tile([P, CB, P], F32, tag="ys")
        for ch in range(CB):
            nc.vector.tensor_add(ys[:, ch, :], tp[:, ch * P : (ch + 1) * P], xrts[ch])
        nc.sync.dma_start(
            out=self.y[b].rearrange("t p n -> p t n")[:, :, ib * P : (ib + 1) * P],
            in_=ys,
        )

    def emit_stage(self, a, b_, extra=None):
        """One pipeline stage: exp-chunk `a` interleaved with outT-chunk `b_`."""
        for g in range(IBT):
            if b_ is not None:
                self.emit_outT_accum(b_[0], b_[1], g)
            if a is not None:
                for jj in range(3 * g, 3 * g + 3):
                    self.emit_scores_exp_j(a[0], a[1], jj)
            if extra == "qk1":
                self.emit_qk_chunk(1, g)
            elif extra == "vt1":
                for nb in range(3 * g, 3 * g + 3):
                    self.emit_vt_one(1, nb)
            if b_ is not None and len(self.pending) >= 2:
                self.emit_epi()


def _build_nc():
    return _Builder().build()


_CACHE = {}


def kernel(x, Wq, bq, Wk, bk, Wv, bv, gamma):
    x = np.asarray(x, dtype=np.float32)
    Wq = np.asarray(Wq, dtype=np.float32)
    bq = np.asarray(bq, dtype=np.float32)
    Wk = np.asarray(Wk, dtype=np.float32)
    bk = np.asarray(bk, dtype=np.float32)
    Wv = np.asarray(Wv, dtype=np.float32)
    bv = np.asarray(bv, dtype=np.float32)
    gamma = np.asarray(gamma, dtype=np.float32)
    g = float(gamma[0])

    xfull = x.reshape(B, C, N)
    # (B, C, N) -> (NCORES, BPC, CB, P, N)
    xblk = xfull.reshape(NCORES, BPC, CB, P, N)

    def chan_block(w):  # (C, K) -> (P, CB, K), partition-major channel blocking
        ck = w.shape[1]
        return np.ascontiguousarray(w.reshape(CB, P, ck).transpose(1, 0, 2))

    wq_h = chan_block(Wq.T).astype(ml_dtypes.bfloat16)          # (P, CB, KC)
    wk_h = chan_block(Wk.T).astype(ml_dtypes.bfloat16)          # (P, CB, KC)
    wv_h = chan_block((Wv * g).T).astype(ml_dtypes.bfloat16)    # (P, CB, C): (g*Wv)^T
    bq_h = np.ascontiguousarray(bq.reshape(KC, 1))
    bk_h = np.ascontiguousarray(bk.reshape(KC, 1))
    bv_h = (bv * g).reshape(1, C).astype(ml_dtypes.bfloat16)

    if "nc" not in _CACHE:
        _CACHE["nc"] = _build_nc()
    nc = _CACHE["nc"]

    in_maps = []
    for core in range(NCORES):
        xc = np.ascontiguousarray(xblk[core])
        in_maps.append(
            {
                "xb": xc.astype(ml_dtypes.bfloat16),
                "xf": xc,
                "wq": wq_h,
                "wk": wk_h,
                "wv": wv_h,
                "bq": bq_h,
                "bk": bk_h,
                "bv": bv_h,
            }
        )

    res = run_bass_kernel_spmd(nc, in_maps, core_ids=list(range(NCORES)))
    out = np.stack([res.results[i]["y"] for i in range(NCORES)])
    # (NCORES, BPC, CB, P, N) -> (B, C, H, W)
    return np.ascontiguousarray(out.reshape(B, C, H, W))


if __name__ == "__main__":
    t0 = time.time()
    nc = _build_nc()
    print(f"build ok: {time.time() - t0:.1f}s")
